# revision 21
# baseline (speedup 1.0000x reference)
"""Trainium2 Bass kernel for nn_BoundaryControlledMixer (4-layer Mamba stack +
boundary-controlled gate), tensor-parallel over d_inner across 8 NeuronCores.

v3: scan-based SSM.  Per core (owns E_loc = 192 of E = 1536 channels):
  - Activations feature-major [feat, token]; projections chain on the PE.
  - SSM state h[e,n] computed exactly via DVE tensor_tensor_scan along the
    token (free) axis: state = dA * state + dBx per partition (fp32 state).
    Layout: 24 tiles of 128 partitions, partition p = (esub, n) with
    esub = p // 16 (8 channels/tile), n = p % 16.
    dA  = exp(a_n * dt[e,t])   (scalar engine, per-partition scale avec)
    dBx = (dt*xc)[e,t] * B[n,t]
    Per-e rows are replicated across n by a PE matmul with the constant
    selector REP8 [8,128] (bf16 PSUM, read directly by scalar/DVE);
    B/C row replication via 8 small SBUF DMAs once per layer.
    y[e,t] = sum_n C[n,t]*h[(e,n),t] via PE matmul with selector GS [128,8],
    16 tiles packed into one [128,T] bf16 PSUM tile, then one DVE copy.
  - Final boundary-gate stage is token-sharded: the last layer's out_proj
    partials (+ residual/8, folded in-PSUM via a scaled-identity matmul) are
    ReduceScattered over tokens; each core finishes LN/gate/mix/out-LN for
    its 256 tokens; the host reassembles.  The gate MLP itself only needs
    x and boundary_prob, so it runs up front, overlapped with the layers.
"""

import numpy as np

import concourse.bacc as bacc
import concourse.bass as bass
import concourse.mybir as mybir
import concourse.tile as tile
from concourse import masks
from concourse.bass_utils import run_bass_kernel_spmd

FP32 = mybir.dt.float32
BF16 = mybir.dt.bfloat16
AF = mybir.ActivationFunctionType
OP = mybir.AluOpType
AX = mybir.AxisListType

B, L, DM, NL = 2, 1024, 768, 4
E, N, K, R = 2 * DM, 16, 4, DM // 16
NC = 8
ELOC = E // NC            # 192
T = B * L                 # 2048
EPS = 1e-5
DTILES = DM // 128        # 6
LPD = L + 2 * K           # padded per-batch xp row
NTIL = ELOC * N // 128    # 24 scan tiles
TLOC = T // NC            # 256 tokens per core (final stage)

_CACHE = {}


def _etiles():
    return [(0, 128), (128, 64)]


def _build():
    nc = bacc.Bacc("TRN2", target_bir_lowering=False, debug=False)

    x_d = nc.dram_tensor("x", [T, DM], FP32, kind="ExternalInput")
    xs_d = nc.dram_tensor("xs", [TLOC, DM], FP32, kind="ExternalInput")
    bps_d = nc.dram_tensor("bps", [1, TLOC], BF16, kind="ExternalInput")
    w_in_d = nc.dram_tensor("w_in", [NL, 128, 6 * 2 * ELOC], BF16, kind="ExternalInput")
    conv_w_d = nc.dram_tensor("conv_w", [NL, 128, 2 * K], FP32, kind="ExternalInput")
    conv_b_d = nc.dram_tensor("conv_b", [NL, 128, 2], FP32, kind="ExternalInput")
    w_xp_d = nc.dram_tensor("w_xp", [NL, 128, 2 * (R + 2 * N)], BF16, kind="ExternalInput")
    w_dt_d = nc.dram_tensor("w_dt", [NL, 65, ELOC], BF16, kind="ExternalInput")
    w_out_d = nc.dram_tensor("w_out", [NL, 128, 2 * DM], BF16, kind="ExternalInput")
    ln_d = nc.dram_tensor("lnp", [NL, 128, 12], FP32, kind="ExternalInput")
    ssmd_d = nc.dram_tensor("ssmd", [NL, 128, 2], FP32, kind="ExternalInput")
    selk_d = nc.dram_tensor("selk", [128, 16 * 128], BF16, kind="ExternalInput")
    gs32_d = nc.dram_tensor("gs32", [128, 4 * 32], BF16, kind="ExternalInput")
    avec_d = nc.dram_tensor("avec", [128, 1], FP32, kind="ExternalInput")
    w1x_d = nc.dram_tensor("w1x", [128, 6 * DM], BF16, kind="ExternalInput")
    w1b_d = nc.dram_tensor("w1b", [1, DM], BF16, kind="ExternalInput")
    b1c_d = nc.dram_tensor("b1c", [128, 6], FP32, kind="ExternalInput")
    w2t_d = nc.dram_tensor("w2t", [128, 6 * DM], BF16, kind="ExternalInput")
    nreps_d = nc.dram_tensor("nreps", [128, 4 * DM], BF16, kind="ExternalInput")

    out_d = nc.dram_tensor("out", [TLOC, DM], FP32, kind="ExternalOutput")
    gate_d = nc.dram_tensor("gate", [TLOC, DM], FP32, kind="ExternalOutput")

    with tile.TileContext(nc) as tc:
        with tc.tile_pool(name="const", bufs=1) as constp, \
             tc.tile_pool(name="persist", bufs=1) as pers, \
             tc.tile_pool(name="wts", bufs=1) as wpool, \
             tc.tile_pool(name="act", bufs=1) as actp, \
             tc.tile_pool(name="st2", bufs=1) as st2, \
             tc.tile_pool(name="vol", bufs=2) as volp, \
             tc.tile_pool(name="ps_mm", bufs=2, space="PSUM") as ps_mm, \
             tc.tile_pool(name="ps_rep", bufs=1, space="PSUM") as ps_rep, \
             tc.tile_pool(name="ps_y", bufs=1, space="PSUM") as ps_y, \
             tc.tile_pool(name="dram", bufs=2, space="DRAM") as dramp:

            def pmm(shape, dt=FP32):
                return ps_mm.tile(shape, dt, name="pmm", tag="pmm")

            # ---------- constants ----------
            ident32 = constp.tile([128, 128], FP32)
            masks.make_identity(nc, ident32[:])
            ident16 = constp.tile([128, 128], BF16)
            masks.make_identity(nc, ident16[:])
            idt8 = constp.tile([128, 128], FP32)
            nc.scalar.activation(idt8[:], ident32[:], AF.Copy, scale=1.0 / NC)
            halfcol32t = constp.tile([128, 1], FP32)
            nc.gpsimd.memset(halfcol32t[:], 0.5)
            halfcol32 = halfcol32t[:, 0:1]
            onesrow16 = constp.tile([1, 128], BF16)
            nc.gpsimd.memset(onesrow16[:], 1.0)
            halfcol16 = constp.tile([128, 1], BF16)
            nc.gpsimd.memset(halfcol16[:], 0.5)
            eps_ap = constp.tile([128, 1], FP32)
            nc.gpsimd.memset(eps_ap[:], EPS)
            selk = constp.tile([128, 16 * 128], BF16)
            nc.sync.dma_start(selk[:], selk_d[:])
            gs32 = constp.tile([128, 4 * 32], BF16)
            nc.sync.dma_start(gs32[:], gs32_d[:])
            avec = constp.tile([128, 1], FP32)
            nc.sync.dma_start(avec[:], avec_d[:])
            nreps = constp.tile([128, 4 * DM], BF16)
            nc.sync.dma_start(nreps[:], nreps_d[:])

            # ---------- gate MLP on this core's 256 tokens (layer-independent) ----------
            w1x = wpool.tile([128, 6 * DM], BF16, name="w1x", tag="w_in_sb")
            nc.sync.dma_start(w1x[:], w1x_d[:])
            w1b = wpool.tile([1, DM], BF16, name="w1b", tag="w1b")
            nc.sync.dma_start(w1b[:], w1b_d[:])
            b1c = wpool.tile([128, 6], FP32, name="b1c", tag="b1c")
            nc.sync.dma_start(b1c[:], b1c_d[:])
            w2t = wpool.tile([128, 6 * DM], BF16, name="w2t", tag="w_out_sb")
            nc.sync.dma_start(w2t[:], w2t_d[:])

            xs_fm = [st2.tile([128, TLOC], BF16, name=f"xs_fm{j}", tag=f"hlnf{j}")
                     for j in range(DTILES)]
            for i in range(2):
                xs_tm = st2.tile([128, DM], FP32, name="xs_tm", tag="x_tm_c", bufs=1)
                nc.sync.dma_start(xs_tm[:], xs_d[i * 128:(i + 1) * 128, :])
                for j in range(DTILES):
                    ptt = pmm([128, 128])
                    nc.tensor.transpose(ptt[:], xs_tm[:, j * 128:(j + 1) * 128],
                                        ident32[:])
                    nc.scalar.copy(xs_fm[j][:, i * 128:(i + 1) * 128], ptt[:])
            brow = actp.tile([1, TLOC], BF16, name="brow", tag="brow")
            nc.sync.dma_start(brow[:], bps_d[:])
            h1g = actp.tile([128, 6 * TLOC], BF16, name="h1g", tag="crep")
            for mt in range(DTILES):
                pt = pmm([128, TLOC])
                for kt in range(DTILES):
                    nc.tensor.matmul(pt[:], w1x[:, kt * DM + mt * 128:kt * DM + (mt + 1) * 128],
                                     xs_fm[kt][:], start=(kt == 0), stop=False)
                nc.tensor.matmul(pt[:], w1b[0:1, mt * 128:(mt + 1) * 128], brow[:],
                                 start=False, stop=True)
                nc.scalar.activation(h1g[:, mt * TLOC:(mt + 1) * TLOC], pt[:],
                                     AF.Silu, bias=b1c[:, mt:mt + 1])
            gate_fm = actp.tile([128, 6 * TLOC], BF16, name="gate_fm", tag="brep")
            for mt in range(DTILES):
                pt = pmm([128, TLOC])
                for kt in range(DTILES):
                    nc.tensor.matmul(pt[:], w2t[:, kt * DM + mt * 128:kt * DM + (mt + 1) * 128],
                                     h1g[:, kt * TLOC:(kt + 1) * TLOC],
                                     start=(kt == 0), stop=(kt == DTILES - 1))
                nc.scalar.activation(gate_fm[:, mt * TLOC:(mt + 1) * TLOC], pt[:], AF.Sigmoid)
            for i in range(2):
                gtm = st2.tile([128, DM], FP32, name="gtm", tag="x_tm_c", bufs=1)
                for j in range(DTILES):
                    ptt = pmm([128, 128], BF16)
                    nc.tensor.transpose(ptt[:], gate_fm[:, j * TLOC + i * 128:j * TLOC + (i + 1) * 128],
                                        ident16[:])
                    nc.scalar.copy(gtm[:, j * 128:(j + 1) * 128], ptt[:])
                nc.sync.dma_start(gate_d[i * 128:(i + 1) * 128, :], gtm[:])

            # ---------- x -> feature-major fp32 residual ----------
            residual = [pers.tile([128, T], FP32, name=f"res{j}") for j in range(DTILES)]
            for c in range(T // 128):
                x_tm_c = st2.tile([128, DM], FP32, name="x_tm_c", tag="x_tm_c", bufs=1)
                nc.sync.dma_start(x_tm_c[:], x_d[c * 128:(c + 1) * 128, :])
                for j in range(DTILES):
                    pt = pmm([128, 128])
                    nc.tensor.transpose(pt[:], x_tm_c[:, j * 128:(j + 1) * 128], ident32[:])
                    nc.scalar.copy(residual[j][:, c * 128:(c + 1) * 128], pt[:])

            # ---------- fused feature-major LayerNorm ----------
            def ln_fm(lnw_aps, lnb_aps, consume):
                for f in range(T // 512):
                    fs = slice(f * 512, (f + 1) * 512)
                    sp1 = pmm([1, 512])
                    sp2 = pmm([1, 512])
                    for j in range(DTILES):
                        nc.tensor.matmul(sp1[:], halfcol32, residual[j][:, fs],
                                         start=(j == 0), stop=(j == DTILES - 1))
                    st1 = st2.tile([1, 512], BF16, name="st1", tag="st1", bufs=2)
                    nc.scalar.activation(st1[:], sp1[:], AF.Copy, scale=2.0 / DM)
                    for j in range(DTILES):
                        sqj = st2.tile([128, 512], BF16, name="ln_sqj", tag="ln_sqj", bufs=2)
                        nc.vector.tensor_tensor(sqj[:], residual[j][:, fs],
                                                residual[j][:, fs], OP.mult)
                        nc.tensor.matmul(sp2[:], halfcol16[:], sqj[:],
                                         start=(j == 0), stop=(j == DTILES - 1))
                    st1b = st2.tile([1, 512], BF16, name="st1b", tag="st1b", bufs=2)
                    nc.scalar.activation(st1b[:], sp2[:], AF.Copy, scale=2.0 / DM)
                    rp = pmm([128, 512])
                    nc.tensor.matmul(rp[:], onesrow16[:1, :128], st1[:],
                                     start=True, stop=True)
                    meanr = st2.tile([128, 512], FP32, name="ln_meanr", tag="ln_meanr", bufs=1)
                    nc.scalar.copy(meanr[:], rp[:])
                    rp2 = pmm([128, 512])
                    nc.tensor.matmul(rp2[:], onesrow16[:1, :128], st1b[:],
                                     start=True, stop=True)
                    invr = st2.tile([128, 512], FP32, name="ln_invr", tag="ln_invr", bufs=1)
                    nc.scalar.copy(invr[:], rp2[:])
                    c2r = st2.tile([128, 512], FP32, name="ln_c2r", tag="ln_c2r", bufs=1)
                    nc.vector.tensor_tensor(c2r[:], meanr[:], meanr[:], OP.mult)
                    nc.vector.tensor_tensor(invr[:], invr[:], c2r[:], OP.subtract)
                    nc.scalar.activation(invr[:], invr[:], AF.Ln, bias=eps_ap[:])
                    nc.scalar.activation(invr[:], invr[:], AF.Exp, scale=-0.5)
                    nc.vector.tensor_tensor(c2r[:], meanr[:], invr[:], OP.mult)
                    slices = []
                    for j in range(DTILES):
                        tmp = st2.tile([128, 512], BF16, name="ln_tmp", tag="ln_tmp", bufs=2)
                        nc.vector.tensor_tensor(tmp[:], residual[j][:, fs], invr[:], OP.mult)
                        nc.vector.tensor_tensor(tmp[:], tmp[:], c2r[:], OP.subtract)
                        hlnf = st2.tile([128, 512], BF16, name="hlnf", tag=f"hlnf{j}")
                        nc.scalar.activation(hlnf[:], tmp[:], AF.Identity,
                                             scale=lnw_aps[j], bias=lnb_aps[j])
                        slices.append(hlnf)
                    consume(f, slices)

            # ================= layers =================
            for li in range(NL):
                w_in = wpool.tile([128, 6 * 2 * ELOC], BF16, name="w_in_sb", tag="w_in_sb")
                nc.sync.dma_start(w_in[:], w_in_d[li])
                w_cw = wpool.tile([128, 2 * K], FP32, name="w_cw_sb", tag="w_cw_sb")
                nc.sync.dma_start(w_cw[:], conv_w_d[li])
                w_cb = wpool.tile([128, 2], FP32, name="w_cb_sb", tag="w_cb_sb")
                nc.sync.dma_start(w_cb[:], conv_b_d[li])
                w_xp = wpool.tile([128, 2 * (R + 2 * N)], BF16, name="w_xp_sb", tag="w_xp_sb")
                nc.sync.dma_start(w_xp[:], w_xp_d[li])
                w_dt = wpool.tile([65, ELOC], BF16, name="w_dt_sb", tag="w_dt_sb")
                nc.sync.dma_start(w_dt[:], w_dt_d[li])
                w_out = wpool.tile([128, 2 * DM], BF16, name="w_out_sb", tag="w_out_sb")
                nc.sync.dma_start(w_out[:], w_out_d[li])
                w_ln = wpool.tile([128, 12], FP32, name="w_ln_sb", tag="w_ln_sb")
                nc.sync.dma_start(w_ln[:], ln_d[li])
                w_D = wpool.tile([128, 2], FP32, name="w_D_sb", tag="w_D_sb")
                nc.sync.dma_start(w_D[:], ssmd_d[li])

                # ---- LN fused with in_proj ----
                xp_t = [actp.tile([128, B * LPD], BF16, name="xp_pad0", tag="xp_pad0"),
                        actp.tile([64, B * LPD], BF16, name="xp_pad1", tag="xp_pad1")]
                z_t = [actp.tile([128, T], BF16, name="z0", tag="z0"),
                       actp.tile([64, T], BF16, name="z1", tag="z1")]
                for ti in range(2):
                    nc.vector.memset(xp_t[ti][:, 0:K], 0.0)
                    nc.vector.memset(xp_t[ti][:, LPD:LPD + K], 0.0)

                def padcol(fs, fl):
                    b_ = fs // L
                    off = b_ * LPD + K + (fs - b_ * L)
                    return slice(off, off + fl)

                def consume_inproj(f, sl6):
                    fs = f * 512
                    for mt in range(3):
                        pt = pmm([128, 512])
                        for kt in range(DTILES):
                            nc.tensor.matmul(
                                pt[:], w_in[:, kt * 384 + mt * 128:kt * 384 + (mt + 1) * 128],
                                sl6[kt][:], start=(kt == 0), stop=(kt == DTILES - 1))
                        if mt == 0:
                            nc.scalar.copy(xp_t[0][:, padcol(fs, 512)], pt[:])
                        elif mt == 1:
                            nc.scalar.copy(xp_t[1][:, padcol(fs, 512)], pt[0:64, :])
                            nc.scalar.copy(z_t[0][0:64, fs:fs + 512], pt[64:128, :])
                        else:
                            nc.scalar.copy(z_t[0][64:128, fs:fs + 512], pt[0:64, :])
                            nc.scalar.copy(z_t[1][:, fs:fs + 512], pt[64:128, :])

                ln_fm([w_ln[:, 2 * j:2 * j + 1] for j in range(DTILES)],
                      [w_ln[:, 2 * j + 1:2 * j + 2] for j in range(DTILES)],
                      consume_inproj)

                # ---- conv + silu ----
                xc = [actp.tile([128, T], BF16, name="xc0", tag="xc0"),
                      actp.tile([64, T], BF16, name="xc1", tag="xc1")]
                HL = L // 2
                for ti, (eo, el) in enumerate(_etiles()):
                    for b_ in range(B):
                        for hf in range(2):
                            acc = st2.tile([el, HL], FP32, name="cacc",
                                           tag="cacc", bufs=1)
                            cb = b_ * LPD + K + hf * HL
                            nc.vector.tensor_scalar(
                                acc[:], xp_t[ti][:el, cb - 3:cb - 3 + HL],
                                w_cw[0:el, ti * K:ti * K + 1], None, OP.mult)
                            for j in range(1, K):
                                nc.vector.scalar_tensor_tensor(
                                    acc[:], xp_t[ti][:el, cb - 3 + j:cb - 3 + j + HL],
                                    w_cw[0:el, ti * K + j:ti * K + j + 1],
                                    acc[:], OP.mult, OP.add)
                            nc.scalar.activation(
                                xc[ti][:el, b_ * L + hf * HL:b_ * L + (hf + 1) * HL],
                                acc[:], AF.Silu, bias=w_cb[0:el, ti:ti + 1])

                # ---- x_proj partial + AllReduce ----
                dbl_in = dramp.tile([R + 2 * N, T], FP32, name="dbl_in", tag="dbl_in")
                dbl_out = dramp.tile([R + 2 * N, T], FP32, name="dbl_out", tag="dbl_out")
                for f in range(T // 512):
                    fs = slice(f * 512, (f + 1) * 512)
                    pt = pmm([80, 512])
                    for ti, (eo, el) in enumerate(_etiles()):
                        nc.tensor.matmul(pt[:], w_xp[0:el, ti * 80:(ti + 1) * 80],
                                         xc[ti][:el, fs], start=(ti == 0), stop=(ti == 1))
                    dblf = st2.tile([80, 512], FP32, name="dblf", tag="dblf", bufs=2)
                    nc.scalar.copy(dblf[:], pt[:])
                    nc.sync.dma_start(dbl_in[:, fs], dblf[:])
                nc.gpsimd.collective_compute("AllReduce", OP.add,
                                             replica_groups=[list(range(NC))],
                                             ins=[dbl_in[:]], outs=[dbl_out[:]])

                # ---- dt (softplus), B/C rows, replicated scan tiles ----
                # dtbig rows 0-47 = dt_in, 48-63 zero, 64 = ones (bias row;
                # engine partition access must start on a 32-quadrant)
                dtbig = actp.tile([65, T], BF16, name="dtbig", tag="dtbig")
                nc.vector.memset(dtbig[32:64, :], 0.0)
                nc.vector.memset(dtbig[64:65, :], 1.0)
                brep = actp.tile([128, T], BF16, name="brep", tag="brep")
                crep = actp.tile([128, T], BF16, name="crep", tag="crep")
                for hf in range(2):
                    hs = slice(hf * 1024, (hf + 1) * 1024)
                    dtf32 = st2.tile([R, T // 2], FP32, name="dtf32", tag="dblf", bufs=2)
                    nc.sync.dma_start(dtf32[:], dbl_out[0:R, hs])
                    nc.vector.tensor_copy(dtbig[0:R, hs], dtf32[:])
                    bcfb = st2.tile([N, T // 2], FP32, name="bcfb", tag="dblf", bufs=2)
                    nc.sync.dma_start(bcfb[:], dbl_out[R:R + N, hs])
                    nc.vector.tensor_copy(brep[0:N, hs], bcfb[:])
                    bcfc = st2.tile([N, T // 2], FP32, name="bcfc", tag="dblf", bufs=2)
                    nc.sync.dma_start(bcfc[:], dbl_out[R + N:R + 2 * N, hs])
                    nc.vector.tensor_copy(crep[0:N, hs], bcfc[:])
                for m in range(1, 8):
                    nc.sync.dma_start(brep[16 * m:16 * (m + 1), :], brep[0:N, :])
                    nc.sync.dma_start(crep[16 * m:16 * (m + 1), :], crep[0:N, :])

                # sp16 = softplus(dt_proj(dt_in)), u16 = sp16 * xc  (e-major)
                sp16 = [actp.tile([128, T], BF16, name="sp0", tag="xp_pad0"),
                        actp.tile([64, T], BF16, name="sp1", tag="xp_pad1")]
                u16 = [actp.tile([128, T], BF16, name="u0", tag="u0", bufs=2),
                       actp.tile([64, T], BF16, name="u1", tag="u1", bufs=1)]
                for ti, (eo, el) in enumerate(_etiles()):
                    for sb in range(4):
                        ss = slice(sb * 512, (sb + 1) * 512)
                        pd = pmm([el, 512])
                        nc.tensor.matmul(pd[:], w_dt[:, eo:eo + el], dtbig[:, ss],
                                         start=True, stop=True)
                        nc.scalar.activation(sp16[ti][:el, ss], pd[:], AF.Exp)
                        nc.scalar.activation(sp16[ti][:el, ss], sp16[ti][:el, ss],
                                             AF.Ln, bias=1.0)
                    nc.vector.tensor_tensor(u16[ti][:el, :], sp16[ti][:el, :],
                                            xc[ti][:el, :], OP.mult)

                # ---- scan tiles ----
                # y feature-major (reuses u16 buffers; u dead once its tiles done)
                y_fm = [actp.tile([128, T], BF16, name="yfm0", tag="u0", bufs=2),
                        actp.tile([64, T], BF16, name="yfm1", tag="u1", bufs=1)]
                yts = {}
                ytpA = ytpB = None

                def issue_tile(k):
                    ti = 0 if k < 16 else 1
                    el = 128 if ti == 0 else 64
                    j = k if k < 16 else k - 16
                    sel = selk[0:el, j * 128:(j + 1) * 128]
                    dA = volp.tile([128, T], BF16, name="dA", tag="dA", bufs=3)
                    d1 = volp.tile([128, T], BF16, name="d1", tag="d1", bufs=2)
                    for sb in range(4):
                        ss = slice(sb * 512, (sb + 1) * 512)
                        pa = ps_rep.tile([128, 512], FP32, name="pa", tag="pa")
                        nc.tensor.matmul(pa[:], sel, sp16[ti][:el, ss],
                                         start=True, stop=True)
                        nc.scalar.activation(dA[:, ss], pa[:], AF.Exp,
                                             scale=avec[:, 0:1])
                        pu = ps_rep.tile([128, 512], FP32, name="pu", tag="pu")
                        nc.tensor.matmul(pu[:], sel, u16[ti][:el, ss],
                                         start=True, stop=True)
                        nc.scalar.copy(d1[:, ss], pu[:])
                    # d1 = urep * Brep in place
                    nc.vector.tensor_tensor(d1[:], d1[:], brep[:], OP.mult)
                    h = volp.tile([128, T], BF16, name="h", tag="d1", bufs=2)
                    for hb in range(2):
                        hs = slice(hb * 1024, (hb + 1) * 1024)
                        nc.vector.tensor_tensor_scan(h[:, hs], dA[:, hs], d1[:, hs],
                                                     0.0, OP.mult, OP.add)
                    yt = volp.tile([128, T], BF16, name="yt", tag="dA", bufs=3)
                    nc.gpsimd.tensor_tensor(yt[:], h[:], crep[:], OP.mult)
                    yts[k] = yt

                def issue_gsum(k):
                    nonlocal ytpA, ytpB
                    gi = k % 4
                    if gi == 0:
                        ytpA = ps_y.tile([32, 1024], FP32, name="ytpA", tag="ytpA")
                        ytpB = ps_y.tile([32, 1024], FP32, name="ytpB", tag="ytpB")
                    yt = yts.pop(k)
                    for sb in range(4):
                        ps = slice(sb * 512, (sb + 1) * 512)
                        ytp = ytpA if sb < 2 else ytpB
                        pp = slice((sb % 2) * 512, (sb % 2) * 512 + 512)
                        nc.tensor.matmul(ytp[:, pp], gs32[:, gi * 32:(gi + 1) * 32],
                                         yt[:, ps], start=(gi == 0), stop=(gi == 3))
                    if gi == 3:
                        g = k // 4
                        if g < 4:
                            dst = y_fm[0][32 * g:32 * (g + 1), :]
                        else:
                            dst = y_fm[1][32 * (g - 4):32 * (g - 3), :]
                        nc.vector.tensor_copy(dst[:, 0:1024], ytpA[:])
                        nc.vector.tensor_copy(dst[:, 1024:2048], ytpB[:])

                for k in range(NTIL + 1):
                    if k < NTIL:
                        issue_tile(k)
                    if k >= 1:
                        issue_gsum(k - 1)

                # ---- D-term, z-gate ----
                for ti, (eo, el) in enumerate(_etiles()):
                    nc.vector.scalar_tensor_tensor(y_fm[ti][:el, :], xc[ti][:el, :],
                                                   w_D[0:el, ti:ti + 1], y_fm[ti][:el, :],
                                                   OP.mult, OP.add)
                    nc.scalar.activation(z_t[ti][:el, :], z_t[ti][:el, :], AF.Silu)
                    nc.vector.tensor_tensor(y_fm[ti][:el, :], y_fm[ti][:el, :],
                                            z_t[ti][:el, :], OP.mult)

                if li < NL - 1:
                    # ---- out_proj partial + AllReduce + residual update ----
                    op_in = dramp.tile([DM, T], BF16, name="op_in", tag="op_in")
                    op_out = dramp.tile([DM, T], BF16, name="op_out", tag="op_out")
                    for mt in range(DTILES):
                        for f in range(T // 512):
                            fs = slice(f * 512, (f + 1) * 512)
                            pt = pmm([128, 512])
                            for ti, (eo, el) in enumerate(_etiles()):
                                nc.tensor.matmul(
                                    pt[:], w_out[0:el, ti * DM + mt * 128:ti * DM + (mt + 1) * 128],
                                    y_fm[ti][:el, fs], start=(ti == 0), stop=(ti == 1))
                            opf = st2.tile([128, 512], BF16, name="opf", tag="opf")
                            nc.scalar.copy(opf[:], pt[:])
                            nc.sync.dma_start(op_in[mt * 128:(mt + 1) * 128, fs], opf[:])
                    nc.gpsimd.collective_compute("AllReduce", OP.add,
                                                 replica_groups=[list(range(NC))],
                                                 ins=[op_in[:]], outs=[op_out[:]])
                    for j in range(DTILES):
                        for f in range(T // 512):
                            fs = slice(f * 512, (f + 1) * 512)
                            hs_f = st2.tile([128, 512], BF16, name="hs_f", tag="hs_f")
                            nc.sync.dma_start(hs_f[:], op_out[j * 128:(j + 1) * 128, fs])
                            nc.vector.tensor_tensor(residual[j][:, fs], residual[j][:, fs],
                                                    hs_f[:], OP.add)
                else:
                    # ---- last layer: token-major out_proj partial + residual/8,
                    #      ReduceScatter over tokens ----
                    rs_in = dramp.tile([T, DM], BF16, name="rs_in", tag="op_in")
                    rs_out = dramp.tile([TLOC, DM], BF16, name="rs_out", tag="rs_out")
                    for tb in range(T // 128):
                        ts = slice(tb * 128, (tb + 1) * 128)
                        pta = pmm([128, 512])
                        ptb = pmm([128, 256])
                        for pt, do, dl in [(pta, 0, 512), (ptb, 512, 256)]:
                            for ti, (eo, el) in enumerate(_etiles()):
                                nc.tensor.matmul(
                                    pt[:, 0:dl],
                                    y_fm[ti][:el, ts],
                                    w_out[0:el, ti * DM + do:ti * DM + do + dl],
                                    start=(ti == 0), stop=(ti == 1))
                        for j in range(DTILES):
                            pt = pta if j < 4 else ptb
                            co = j * 128 if j < 4 else (j - 4) * 128
                            nc.tensor.matmul(pt[:, co:co + 128],
                                             residual[j][:, ts], idt8[:],
                                             start=False, stop=True,
                                             skip_group_check=True)
                        rf = st2.tile([128, DM], BF16, name="rf", tag="rf")
                        nc.scalar.copy(rf[:, 0:512], pta[:])
                        nc.scalar.copy(rf[:, 512:DM], ptb[:])
                        nc.sync.dma_start(rs_in[ts, :], rf[:])
                    nc.gpsimd.collective_compute("ReduceScatter", OP.add,
                                                 replica_groups=[list(range(NC))],
                                                 ins=[rs_in[:]], outs=[rs_out[:]])

            # ================= final stage (this core's 256 tokens) =================
            nfw = nreps[:, 0:DM]
            nfb = nreps[:, DM:2 * DM]
            olw = nreps[:, 2 * DM:3 * DM]
            olb = nreps[:, 3 * DM:4 * DM]

            def ln_tm(src_ap, wrep, brep_, dst_ap):
                st = st2.tile([128, 1], FP32, name="lnt_st", tag="lnt_st", bufs=1)
                nc.vector.tensor_reduce(st[:], src_ap, axis=AX.X, op=OP.add)
                nc.scalar.activation(st[:], st[:], AF.Copy, scale=1.0 / DM)
                ot = st2.tile([128, DM], FP32, name="lnt_ot", tag="lnt_ot", bufs=1)
                nc.vector.tensor_scalar(ot[:], src_ap, st[:, 0:1], None, OP.subtract)
                sq = st2.tile([128, DM], FP32, name="lnt_sq", tag="lnt_sq", bufs=1)
                nc.vector.tensor_tensor(sq[:], ot[:], ot[:], OP.mult)
                v2 = st2.tile([128, 1], FP32, name="lnt_v2", tag="lnt_v2", bufs=1)
                nc.vector.tensor_reduce(v2[:], sq[:], axis=AX.X, op=OP.add)
                nc.scalar.activation(v2[:], v2[:], AF.Ln, bias=eps_ap[:], scale=1.0 / DM)
                nc.scalar.activation(v2[:], v2[:], AF.Exp, scale=-0.5)
                nc.vector.tensor_scalar(ot[:], ot[:], v2[:, 0:1], None, OP.mult)
                nc.vector.tensor_tensor(ot[:], ot[:], wrep, OP.mult)
                nc.vector.tensor_tensor(dst_ap, ot[:], brep_, OP.add)

            for i in range(2):
                mx = st2.tile([128, DM], BF16, name="mx", tag="opf")
                nc.sync.dma_start(mx[:], rs_out[i * 128:(i + 1) * 128, :])
                xst = st2.tile([128, DM], FP32, name="xst", tag="lnt_xs", bufs=1)
                nc.sync.dma_start(xst[:], xs_d[i * 128:(i + 1) * 128, :])
                gtt = st2.tile([128, DM], FP32, name="gtt", tag="lnt_gt", bufs=1)
                nc.sync.dma_start(gtt[:], gate_d[i * 128:(i + 1) * 128, :])
                mixed = st2.tile([128, DM], FP32, name="mixed", tag="lnt_mx", bufs=1)
                ln_tm(mx[:], nfw, nfb, mixed[:])
                ot2 = st2.tile([128, DM], FP32, name="ot2", tag="lnt_ot2", bufs=1)
                nc.vector.tensor_tensor(ot2[:], mixed[:], xst[:], OP.subtract)
                nc.vector.tensor_tensor(ot2[:], ot2[:], gtt[:], OP.mult)
                nc.vector.tensor_tensor(ot2[:], ot2[:], xst[:], OP.add)
                fin = st2.tile([128, DM], FP32, name="fin", tag="rf", bufs=1)
                ln_tm(ot2[:], olw, olb, fin[:])
                nc.sync.dma_start(out_d[i * 128:(i + 1) * 128, :], fin[:])

    nc.compile()
    return nc


def _pack_fm(arr, pad_to=128):
    arr = np.asarray(arr)
    if arr.ndim == 1:
        arr = arr[:, None]
    F, W = arr.shape
    nblk = (F + pad_to - 1) // pad_to
    outp = np.zeros((pad_to, nblk * W), dtype=arr.dtype)
    for b_ in range(nblk):
        blk = arr[b_ * pad_to:(b_ + 1) * pad_to]
        outp[:blk.shape[0], b_ * W:(b_ + 1) * W] = blk
    return outp


def _prep_inputs(inputs):
    f32 = np.float32
    x = np.ascontiguousarray(np.asarray(inputs["x"], f32).reshape(T, DM))
    bprob = np.ascontiguousarray(np.asarray(inputs["boundary_prob"], f32).reshape(1, T))
    A = -np.exp(np.asarray(inputs["A_log"], f32))
    a_scales = A[0, 0, :]
    # selk[r, j*128+m] = 1 iff r == 8j + m//16  (pick e-row band j, replicate x16)
    selk = np.zeros((128, 16 * 128), f32)
    for j in range(16):
        m = np.arange(128)
        selk[8 * j + m // 16, j * 128 + m] = 1.0
    # gs32[p, i*32+m] = 1 iff m == 8i + p//16  (sum over n into e-row band i)
    gs32 = np.zeros((128, 4 * 32), f32)
    for i in range(4):
        p = np.arange(128)
        gs32[p, i * 32 + 8 * i + p // 16] = 1.0
    avec = a_scales[np.arange(128) % 16][:, None].astype(f32)
    cw1 = np.asarray(inputs["ctrl_w1"], f32)
    w1x = _pack_fm(cw1[:, :DM].T)                       # [128, 6*DM]
    w1b = cw1[:, DM][None, :]                           # [1, DM]
    b1c = _pack_fm(np.asarray(inputs["ctrl_b1"], f32))  # [128, 6]
    w2t = _pack_fm(np.asarray(inputs["ctrl_w2"], f32).T)
    nreps = np.concatenate([
        np.broadcast_to(np.asarray(inputs["normf_w"], f32), (128, DM)),
        np.broadcast_to(np.asarray(inputs["normf_b"], f32), (128, DM)),
        np.broadcast_to(np.asarray(inputs["out_ln_w"], f32), (128, DM)),
        np.broadcast_to(np.asarray(inputs["out_ln_b"], f32), (128, DM)),
    ], axis=1)
    maps = []
    for c in range(NC):
        sl = slice(c * ELOC, (c + 1) * ELOC)
        w_in = np.stack([_pack_fm(
            np.concatenate([np.asarray(inputs["in_proj_w"][i])[sl],
                            np.asarray(inputs["in_proj_w"][i])[E + c * ELOC:E + (c + 1) * ELOC]],
                           axis=0).T.astype(f32))
            for i in range(NL)])
        w_xp = np.stack([_pack_fm(np.asarray(inputs["x_proj_w"][i], f32)[:, sl].T)
                         for i in range(NL)])
        w_dt = np.stack([
            np.concatenate([np.asarray(inputs["dt_proj_w"][i], f32)[sl].T,
                            np.zeros((16, ELOC), f32),
                            np.asarray(inputs["dt_proj_b"][i], f32)[None, sl]], axis=0)
            for i in range(NL)])
        w_out = np.stack([_pack_fm(np.asarray(inputs["out_proj_w"][i], f32)[:, sl].T)
                          for i in range(NL)])
        lnp = np.stack([_pack_fm(np.stack([np.asarray(inputs["ln_w"][i], f32),
                                           np.asarray(inputs["ln_b"][i], f32)], axis=1))
                        for i in range(NL)])
        tsl = slice(c * TLOC, (c + 1) * TLOC)
        maps.append({
            "x": x, "xs": np.ascontiguousarray(x[tsl]),
            "bps": np.ascontiguousarray(bprob[:, tsl]),
            "w_in": w_in,
            "conv_w": np.stack([_pack_fm(np.asarray(inputs["conv_w"][i], f32)[sl])
                                for i in range(NL)]),
            "conv_b": np.stack([_pack_fm(np.asarray(inputs["conv_b"][i], f32)[sl])
                                for i in range(NL)]),
            "w_xp": w_xp, "w_dt": w_dt, "w_out": w_out, "lnp": lnp,
            "ssmd": np.stack([_pack_fm(np.asarray(inputs["ssm_D"][i], f32)[sl])
                              for i in range(NL)]),
            "selk": selk, "gs32": gs32, "avec": avec,
            "w1x": w1x, "w1b": w1b, "b1c": b1c, "w2t": w2t,
            "nreps": nreps,
        })
    return maps


BF16_KEYS = ("w_in", "w_xp", "w_out", "w_dt", "selk", "gs32", "w1x", "w1b",
             "w2t", "bps", "nreps")


def kernel(**inputs):
    import ml_dtypes
    maps = _prep_inputs(inputs)
    A = -np.exp(np.asarray(inputs["A_log"], np.float32))
    a_scales = A[0, 0, :]
    for i in range(NL):
        assert np.allclose(A[i], np.broadcast_to(a_scales, (E, N)), rtol=1e-5, atol=1e-6), \
            "kernel assumes channel-independent A"
    if "nc" not in _CACHE:
        _CACHE["nc"] = _build()
    nc = _CACHE["nc"]
    for m in maps:
        for k in BF16_KEYS:
            m[k] = np.asarray(m[k], dtype=ml_dtypes.bfloat16)
    res = run_bass_kernel_spmd(nc, maps, list(range(NC)))
    kernel._res = res
    out = np.concatenate([np.asarray(res.results[c]["out"], np.float32)
                          for c in range(NC)], axis=0).reshape(B, L, DM)
    gate = np.concatenate([np.asarray(res.results[c]["gate"], np.float32)
                           for c in range(NC)], axis=0).reshape(B, L, DM)
    return out, gate


# revision 22
# speedup vs baseline: 1.0595x; 1.0595x over previous
"""Trainium2 Bass kernel for nn_BoundaryControlledMixer (4-layer Mamba stack +
boundary-controlled gate), tensor-parallel over d_inner across 8 NeuronCores.

v3: scan-based SSM.  Per core (owns E_loc = 192 of E = 1536 channels):
  - Activations feature-major [feat, token]; projections chain on the PE.
  - SSM state h[e,n] computed exactly via DVE tensor_tensor_scan along the
    token (free) axis: state = dA * state + dBx per partition (fp32 state).
    Layout: 24 tiles of 128 partitions, partition p = (esub, n) with
    esub = p // 16 (8 channels/tile), n = p % 16.
    dA  = exp(a_n * dt[e,t])   (scalar engine, per-partition scale avec)
    dBx = (dt*xc)[e,t] * B[n,t]
    Per-e rows are replicated across n by a PE matmul with the constant
    selector REP8 [8,128] (bf16 PSUM, read directly by scalar/DVE);
    B/C row replication via 8 small SBUF DMAs once per layer.
    y[e,t] = sum_n C[n,t]*h[(e,n),t] via PE matmul with selector GS [128,8],
    16 tiles packed into one [128,T] bf16 PSUM tile, then one DVE copy.
  - Final boundary-gate stage is token-sharded: the last layer's out_proj
    partials (+ residual/8, folded in-PSUM via a scaled-identity matmul) are
    ReduceScattered over tokens; each core finishes LN/gate/mix/out-LN for
    its 256 tokens; the host reassembles.  The gate MLP itself only needs
    x and boundary_prob, so it runs up front, overlapped with the layers.
"""

import numpy as np

import concourse.bacc as bacc
import concourse.bass as bass
import concourse.mybir as mybir
import concourse.tile as tile
from concourse import masks
from concourse.bass_utils import run_bass_kernel_spmd

FP32 = mybir.dt.float32
BF16 = mybir.dt.bfloat16
AF = mybir.ActivationFunctionType
OP = mybir.AluOpType
AX = mybir.AxisListType

B, L, DM, NL = 2, 1024, 768, 4
E, N, K, R = 2 * DM, 16, 4, DM // 16
NC = 8
ELOC = E // NC            # 192
T = B * L                 # 2048
EPS = 1e-5
DTILES = DM // 128        # 6
LPD = L + 2 * K           # padded per-batch xp row
NTIL = ELOC * N // 128    # 24 scan tiles
TLOC = T // NC            # 256 tokens per core (final stage)

_CACHE = {}


def _etiles():
    return [(0, 128), (128, 64)]


def _build():
    nc = bacc.Bacc("TRN2", target_bir_lowering=False, debug=False)

    x_d = nc.dram_tensor("x", [T, DM], FP32, kind="ExternalInput")
    xs_d = nc.dram_tensor("xs", [TLOC, DM], FP32, kind="ExternalInput")
    bps_d = nc.dram_tensor("bps", [1, TLOC], BF16, kind="ExternalInput")
    w_in_d = nc.dram_tensor("w_in", [NL, 128, 6 * 2 * ELOC], BF16, kind="ExternalInput")
    conv_w_d = nc.dram_tensor("conv_w", [NL, 128, 2 * K], FP32, kind="ExternalInput")
    conv_b_d = nc.dram_tensor("conv_b", [NL, 128, 2], FP32, kind="ExternalInput")
    w_xp_d = nc.dram_tensor("w_xp", [NL, 128, 2 * (R + 2 * N)], BF16, kind="ExternalInput")
    w_dt_d = nc.dram_tensor("w_dt", [NL, 65, ELOC], BF16, kind="ExternalInput")
    w_out_d = nc.dram_tensor("w_out", [NL, 128, 2 * DM], BF16, kind="ExternalInput")
    ln_d = nc.dram_tensor("lnp", [NL, 128, 12], FP32, kind="ExternalInput")
    ssmd_d = nc.dram_tensor("ssmd", [NL, 128, 2], FP32, kind="ExternalInput")
    selk_d = nc.dram_tensor("selk", [128, 16 * 128], BF16, kind="ExternalInput")
    gs32_d = nc.dram_tensor("gs32", [128, 4 * 32], BF16, kind="ExternalInput")
    avec_d = nc.dram_tensor("avec", [128, 1], FP32, kind="ExternalInput")
    w1x_d = nc.dram_tensor("w1x", [128, 6 * DM], BF16, kind="ExternalInput")
    w1b_d = nc.dram_tensor("w1b", [1, DM], BF16, kind="ExternalInput")
    b1c_d = nc.dram_tensor("b1c", [128, 6], FP32, kind="ExternalInput")
    w2t_d = nc.dram_tensor("w2t", [128, 6 * DM], BF16, kind="ExternalInput")
    nreps_d = nc.dram_tensor("nreps", [128, 4 * DM], BF16, kind="ExternalInput")

    out_d = nc.dram_tensor("out", [TLOC, DM], FP32, kind="ExternalOutput")
    gate_d = nc.dram_tensor("gate", [TLOC, DM], FP32, kind="ExternalOutput")

    with tile.TileContext(nc) as tc:
        with tc.tile_pool(name="const", bufs=1) as constp, \
             tc.tile_pool(name="persist", bufs=1) as pers, \
             tc.tile_pool(name="wts", bufs=1) as wpool, \
             tc.tile_pool(name="act", bufs=1) as actp, \
             tc.tile_pool(name="st2", bufs=1) as st2, \
             tc.tile_pool(name="vol", bufs=2) as volp, \
             tc.tile_pool(name="ps_mm", bufs=2, space="PSUM") as ps_mm, \
             tc.tile_pool(name="ps_rep", bufs=1, space="PSUM") as ps_rep, \
             tc.tile_pool(name="ps_y", bufs=1, space="PSUM") as ps_y, \
             tc.tile_pool(name="dram", bufs=2, space="DRAM") as dramp:

            def pmm(shape, dt=FP32):
                return ps_mm.tile(shape, dt, name="pmm", tag="pmm")

            # ---------- constants ----------
            ident32 = constp.tile([128, 128], FP32)
            masks.make_identity(nc, ident32[:])
            ident16 = constp.tile([128, 128], BF16)
            masks.make_identity(nc, ident16[:])
            idt8 = constp.tile([128, 128], FP32)
            nc.scalar.activation(idt8[:], ident32[:], AF.Copy, scale=1.0 / NC)
            halfcol32t = constp.tile([128, 1], FP32)
            nc.gpsimd.memset(halfcol32t[:], 0.5)
            halfcol32 = halfcol32t[:, 0:1]
            onesrow16 = constp.tile([1, 128], BF16)
            nc.gpsimd.memset(onesrow16[:], 1.0)
            halfcol16 = constp.tile([128, 1], BF16)
            nc.gpsimd.memset(halfcol16[:], 0.5)
            eps_ap = constp.tile([128, 1], FP32)
            nc.gpsimd.memset(eps_ap[:], EPS)
            selk = constp.tile([128, 16 * 128], BF16)
            nc.sync.dma_start(selk[:], selk_d[:])
            gs32 = constp.tile([128, 4 * 32], BF16)
            nc.sync.dma_start(gs32[:], gs32_d[:])
            avec = constp.tile([128, 1], FP32)
            nc.sync.dma_start(avec[:], avec_d[:])
            nreps = constp.tile([128, 4 * DM], BF16)
            nc.sync.dma_start(nreps[:], nreps_d[:])

            # ---------- gate MLP on this core's 256 tokens (layer-independent) ----------
            w1x = wpool.tile([128, 6 * DM], BF16, name="w1x", tag="w_in_sb")
            nc.sync.dma_start(w1x[:], w1x_d[:])
            w1b = wpool.tile([1, DM], BF16, name="w1b", tag="w1b")
            nc.sync.dma_start(w1b[:], w1b_d[:])
            b1c = wpool.tile([128, 6], FP32, name="b1c", tag="b1c")
            nc.sync.dma_start(b1c[:], b1c_d[:])
            w2t = wpool.tile([128, 6 * DM], BF16, name="w2t", tag="w_out_sb")
            nc.sync.dma_start(w2t[:], w2t_d[:])

            xs_fm = [st2.tile([128, TLOC], BF16, name=f"xs_fm{j}", tag=f"hlnf{j}")
                     for j in range(DTILES)]
            for i in range(2):
                xs_tm = st2.tile([128, DM], FP32, name="xs_tm", tag="x_tm_c", bufs=1)
                nc.sync.dma_start(xs_tm[:], xs_d[i * 128:(i + 1) * 128, :])
                for j in range(DTILES):
                    ptt = pmm([128, 128])
                    nc.tensor.transpose(ptt[:], xs_tm[:, j * 128:(j + 1) * 128],
                                        ident32[:])
                    nc.scalar.copy(xs_fm[j][:, i * 128:(i + 1) * 128], ptt[:])
            brow = actp.tile([1, TLOC], BF16, name="brow", tag="brow")
            nc.sync.dma_start(brow[:], bps_d[:])
            h1g = actp.tile([128, 6 * TLOC], BF16, name="h1g", tag="crep")
            for mt in range(DTILES):
                pt = pmm([128, TLOC])
                for kt in range(DTILES):
                    nc.tensor.matmul(pt[:], w1x[:, kt * DM + mt * 128:kt * DM + (mt + 1) * 128],
                                     xs_fm[kt][:], start=(kt == 0), stop=False)
                nc.tensor.matmul(pt[:], w1b[0:1, mt * 128:(mt + 1) * 128], brow[:],
                                 start=False, stop=True)
                nc.scalar.activation(h1g[:, mt * TLOC:(mt + 1) * TLOC], pt[:],
                                     AF.Silu, bias=b1c[:, mt:mt + 1])
            gate_fm = actp.tile([128, 6 * TLOC], BF16, name="gate_fm", tag="brep")
            for mt in range(DTILES):
                pt = pmm([128, TLOC])
                for kt in range(DTILES):
                    nc.tensor.matmul(pt[:], w2t[:, kt * DM + mt * 128:kt * DM + (mt + 1) * 128],
                                     h1g[:, kt * TLOC:(kt + 1) * TLOC],
                                     start=(kt == 0), stop=(kt == DTILES - 1))
                nc.scalar.activation(gate_fm[:, mt * TLOC:(mt + 1) * TLOC], pt[:], AF.Sigmoid)
            for i in range(2):
                gtm = st2.tile([128, DM], FP32, name="gtm", tag="x_tm_c", bufs=1)
                for j in range(DTILES):
                    ptt = pmm([128, 128], BF16)
                    nc.tensor.transpose(ptt[:], gate_fm[:, j * TLOC + i * 128:j * TLOC + (i + 1) * 128],
                                        ident16[:])
                    nc.scalar.copy(gtm[:, j * 128:(j + 1) * 128], ptt[:])
                nc.sync.dma_start(gate_d[i * 128:(i + 1) * 128, :], gtm[:])

            # ---------- x -> feature-major fp32 residual ----------
            residual = [pers.tile([128, T], FP32, name=f"res{j}") for j in range(DTILES)]
            for c in range(T // 128):
                x_tm_c = st2.tile([128, DM], FP32, name="x_tm_c", tag="x_tm_c", bufs=1)
                nc.sync.dma_start(x_tm_c[:], x_d[c * 128:(c + 1) * 128, :])
                for j in range(DTILES):
                    pt = pmm([128, 128])
                    nc.tensor.transpose(pt[:], x_tm_c[:, j * 128:(j + 1) * 128], ident32[:])
                    nc.scalar.copy(residual[j][:, c * 128:(c + 1) * 128], pt[:])

            # ---------- fused feature-major LayerNorm ----------
            def ln_fm(lnw_aps, lnb_aps, consume):
                for f in range(T // 512):
                    fs = slice(f * 512, (f + 1) * 512)
                    sp1 = pmm([1, 512])
                    sp2 = pmm([1, 512])
                    for j in range(DTILES):
                        nc.tensor.matmul(sp1[:], halfcol32, residual[j][:, fs],
                                         start=(j == 0), stop=(j == DTILES - 1))
                    st1 = st2.tile([1, 512], BF16, name="st1", tag="st1", bufs=2)
                    nc.scalar.activation(st1[:], sp1[:], AF.Copy, scale=2.0 / DM)
                    for j in range(DTILES):
                        sqj = st2.tile([128, 512], BF16, name="ln_sqj", tag="ln_sqj", bufs=2)
                        nc.vector.tensor_tensor(sqj[:], residual[j][:, fs],
                                                residual[j][:, fs], OP.mult)
                        nc.tensor.matmul(sp2[:], halfcol16[:], sqj[:],
                                         start=(j == 0), stop=(j == DTILES - 1))
                    st1b = st2.tile([1, 512], BF16, name="st1b", tag="st1b", bufs=2)
                    nc.scalar.activation(st1b[:], sp2[:], AF.Copy, scale=2.0 / DM)
                    rp = pmm([128, 512])
                    nc.tensor.matmul(rp[:], onesrow16[:1, :128], st1[:],
                                     start=True, stop=True)
                    meanr = st2.tile([128, 512], FP32, name="ln_meanr", tag="ln_meanr", bufs=1)
                    nc.scalar.copy(meanr[:], rp[:])
                    rp2 = pmm([128, 512])
                    nc.tensor.matmul(rp2[:], onesrow16[:1, :128], st1b[:],
                                     start=True, stop=True)
                    invr = st2.tile([128, 512], FP32, name="ln_invr", tag="ln_invr", bufs=1)
                    nc.scalar.copy(invr[:], rp2[:])
                    c2r = st2.tile([128, 512], FP32, name="ln_c2r", tag="ln_c2r", bufs=1)
                    nc.vector.tensor_tensor(c2r[:], meanr[:], meanr[:], OP.mult)
                    nc.vector.tensor_tensor(invr[:], invr[:], c2r[:], OP.subtract)
                    nc.scalar.activation(invr[:], invr[:], AF.Ln, bias=eps_ap[:])
                    nc.scalar.activation(invr[:], invr[:], AF.Exp, scale=-0.5)
                    nc.vector.tensor_tensor(c2r[:], meanr[:], invr[:], OP.mult)
                    slices = []
                    for j in range(DTILES):
                        tmp = st2.tile([128, 512], BF16, name="ln_tmp", tag="ln_tmp", bufs=2)
                        nc.vector.tensor_tensor(tmp[:], residual[j][:, fs], invr[:], OP.mult)
                        nc.vector.tensor_tensor(tmp[:], tmp[:], c2r[:], OP.subtract)
                        hlnf = st2.tile([128, 512], BF16, name="hlnf", tag=f"hlnf{j}")
                        nc.scalar.activation(hlnf[:], tmp[:], AF.Identity,
                                             scale=lnw_aps[j], bias=lnb_aps[j])
                        slices.append(hlnf)
                    consume(f, slices)

            # ================= layers =================
            for li in range(NL):
                w_in = wpool.tile([128, 6 * 2 * ELOC], BF16, name="w_in_sb", tag="w_in_sb")
                nc.sync.dma_start(w_in[:], w_in_d[li])
                w_cw = wpool.tile([128, 2 * K], FP32, name="w_cw_sb", tag="w_cw_sb")
                nc.sync.dma_start(w_cw[:], conv_w_d[li])
                w_cb = wpool.tile([128, 2], FP32, name="w_cb_sb", tag="w_cb_sb")
                nc.sync.dma_start(w_cb[:], conv_b_d[li])
                w_xp = wpool.tile([128, 2 * (R + 2 * N)], BF16, name="w_xp_sb", tag="w_xp_sb")
                nc.sync.dma_start(w_xp[:], w_xp_d[li])
                w_dt = wpool.tile([65, ELOC], BF16, name="w_dt_sb", tag="w_dt_sb")
                nc.sync.dma_start(w_dt[:], w_dt_d[li])
                w_out = wpool.tile([128, 2 * DM], BF16, name="w_out_sb", tag="w_out_sb")
                nc.sync.dma_start(w_out[:], w_out_d[li])
                w_ln = wpool.tile([128, 12], FP32, name="w_ln_sb", tag="w_ln_sb")
                nc.sync.dma_start(w_ln[:], ln_d[li])
                w_D = wpool.tile([128, 2], FP32, name="w_D_sb", tag="w_D_sb")
                nc.sync.dma_start(w_D[:], ssmd_d[li])

                # ---- LN fused with in_proj ----
                xp_t = [actp.tile([128, B * LPD], BF16, name="xp_pad0", tag="xp_pad0"),
                        actp.tile([64, B * LPD], BF16, name="xp_pad1", tag="xp_pad1")]
                z_t = [actp.tile([128, T], BF16, name="z0", tag="z0"),
                       actp.tile([64, T], BF16, name="z1", tag="z1")]
                for ti in range(2):
                    nc.vector.memset(xp_t[ti][:, 0:K], 0.0)
                    nc.vector.memset(xp_t[ti][:, LPD:LPD + K], 0.0)

                def padcol(fs, fl):
                    b_ = fs // L
                    off = b_ * LPD + K + (fs - b_ * L)
                    return slice(off, off + fl)

                def consume_inproj(f, sl6):
                    fs = f * 512
                    for mt in range(3):
                        pt = pmm([128, 512])
                        for kt in range(DTILES):
                            nc.tensor.matmul(
                                pt[:], w_in[:, kt * 384 + mt * 128:kt * 384 + (mt + 1) * 128],
                                sl6[kt][:], start=(kt == 0), stop=(kt == DTILES - 1))
                        if mt == 0:
                            nc.scalar.copy(xp_t[0][:, padcol(fs, 512)], pt[:])
                        elif mt == 1:
                            nc.scalar.copy(xp_t[1][:, padcol(fs, 512)], pt[0:64, :])
                            nc.scalar.copy(z_t[0][0:64, fs:fs + 512], pt[64:128, :])
                        else:
                            nc.scalar.copy(z_t[0][64:128, fs:fs + 512], pt[0:64, :])
                            nc.scalar.copy(z_t[1][:, fs:fs + 512], pt[64:128, :])

                ln_fm([w_ln[:, 2 * j:2 * j + 1] for j in range(DTILES)],
                      [w_ln[:, 2 * j + 1:2 * j + 2] for j in range(DTILES)],
                      consume_inproj)

                # ---- conv + silu ----
                xc = [actp.tile([128, T], BF16, name="xc0", tag="xc0"),
                      actp.tile([64, T], BF16, name="xc1", tag="xc1")]
                HL = L // 2
                for ti, (eo, el) in enumerate(_etiles()):
                    for b_ in range(B):
                        for hf in range(2):
                            acc = st2.tile([el, HL], FP32, name="cacc",
                                           tag="cacc", bufs=1)
                            cb = b_ * LPD + K + hf * HL
                            nc.vector.tensor_scalar(
                                acc[:], xp_t[ti][:el, cb - 3:cb - 3 + HL],
                                w_cw[0:el, ti * K:ti * K + 1], None, OP.mult)
                            for j in range(1, K):
                                nc.vector.scalar_tensor_tensor(
                                    acc[:], xp_t[ti][:el, cb - 3 + j:cb - 3 + j + HL],
                                    w_cw[0:el, ti * K + j:ti * K + j + 1],
                                    acc[:], OP.mult, OP.add)
                            nc.scalar.activation(
                                xc[ti][:el, b_ * L + hf * HL:b_ * L + (hf + 1) * HL],
                                acc[:], AF.Silu, bias=w_cb[0:el, ti:ti + 1])

                # ---- x_proj partial + AllReduce ----
                dbl_in = dramp.tile([R + 2 * N, T], FP32, name="dbl_in", tag="dbl_in")
                dbl_out = dramp.tile([R + 2 * N, T], FP32, name="dbl_out", tag="dbl_out")
                for f in range(T // 512):
                    fs = slice(f * 512, (f + 1) * 512)
                    pt = pmm([80, 512])
                    for ti, (eo, el) in enumerate(_etiles()):
                        nc.tensor.matmul(pt[:], w_xp[0:el, ti * 80:(ti + 1) * 80],
                                         xc[ti][:el, fs], start=(ti == 0), stop=(ti == 1))
                    dblf = st2.tile([80, 512], FP32, name="dblf", tag="dblf", bufs=2)
                    nc.scalar.copy(dblf[:], pt[:])
                    nc.sync.dma_start(dbl_in[:, fs], dblf[:])
                nc.gpsimd.collective_compute("AllReduce", OP.add,
                                             replica_groups=[list(range(NC))],
                                             ins=[dbl_in[:]], outs=[dbl_out[:]])

                # ---- dt (softplus), B/C rows, replicated scan tiles ----
                # dtbig rows 0-47 = dt_in, 48-63 zero, 64 = ones (bias row;
                # engine partition access must start on a 32-quadrant)
                dtbig = actp.tile([65, T], BF16, name="dtbig", tag="dtbig")
                nc.vector.memset(dtbig[32:64, :], 0.0)
                nc.vector.memset(dtbig[64:65, :], 1.0)
                brep = actp.tile([128, T], BF16, name="brep", tag="brep")
                crep = actp.tile([128, T], BF16, name="crep", tag="crep")
                for hf in range(2):
                    hs = slice(hf * 1024, (hf + 1) * 1024)
                    dtf32 = st2.tile([R, T // 2], FP32, name="dtf32", tag="dblf", bufs=2)
                    nc.sync.dma_start(dtf32[:], dbl_out[0:R, hs])
                    nc.vector.tensor_copy(dtbig[0:R, hs], dtf32[:])
                    bcfb = st2.tile([N, T // 2], FP32, name="bcfb", tag="dblf", bufs=2)
                    nc.sync.dma_start(bcfb[:], dbl_out[R:R + N, hs])
                    nc.vector.tensor_copy(brep[0:N, hs], bcfb[:])
                    bcfc = st2.tile([N, T // 2], FP32, name="bcfc", tag="dblf", bufs=2)
                    nc.sync.dma_start(bcfc[:], dbl_out[R + N:R + 2 * N, hs])
                    nc.vector.tensor_copy(crep[0:N, hs], bcfc[:])
                for m in range(1, 8):
                    nc.sync.dma_start(brep[16 * m:16 * (m + 1), :], brep[0:N, :])
                    nc.sync.dma_start(crep[16 * m:16 * (m + 1), :], crep[0:N, :])

                # sp16 = softplus(dt_proj(dt_in)), u16 = sp16 * xc  (e-major)
                sp16 = [actp.tile([128, T], BF16, name="sp0", tag="xp_pad0"),
                        actp.tile([64, T], BF16, name="sp1", tag="xp_pad1")]
                u16 = [actp.tile([128, T], BF16, name="u0", tag="u0", bufs=2),
                       actp.tile([64, T], BF16, name="u1", tag="u1", bufs=1)]
                for ti, (eo, el) in enumerate(_etiles()):
                    for sb in range(4):
                        ss = slice(sb * 512, (sb + 1) * 512)
                        pd = pmm([el, 512])
                        nc.tensor.matmul(pd[:], w_dt[:, eo:eo + el], dtbig[:, ss],
                                         start=True, stop=True)
                        nc.scalar.activation(sp16[ti][:el, ss], pd[:], AF.Exp)
                        nc.scalar.activation(sp16[ti][:el, ss], sp16[ti][:el, ss],
                                             AF.Ln, bias=1.0)
                    nc.vector.tensor_tensor(u16[ti][:el, :], sp16[ti][:el, :],
                                            xc[ti][:el, :], OP.mult)

                # ---- scan tiles ----
                # y feature-major (reuses u16 buffers; u dead once its tiles done)
                y_fm = [actp.tile([128, T], BF16, name="yfm0", tag="u0", bufs=2),
                        actp.tile([64, T], BF16, name="yfm1", tag="u1", bufs=1)]
                yts = {}
                ytpA = ytpB = None

                def issue_tile(k):
                    ti = 0 if k < 16 else 1
                    el = 128 if ti == 0 else 64
                    j = k if k < 16 else k - 16
                    sel = selk[0:el, j * 128:(j + 1) * 128]
                    dA = volp.tile([128, T], BF16, name="dA", tag="dA", bufs=3)
                    d1 = volp.tile([128, T], BF16, name="d1", tag="d1", bufs=2)
                    for sb in range(4):
                        ss = slice(sb * 512, (sb + 1) * 512)
                        pa = ps_rep.tile([128, 512], FP32, name="pa", tag="pa")
                        nc.tensor.matmul(pa[:], sel, sp16[ti][:el, ss],
                                         start=True, stop=True)
                        nc.scalar.activation(dA[:, ss], pa[:], AF.Exp,
                                             scale=avec[:, 0:1])
                        pu = ps_rep.tile([128, 512], FP32, name="pu", tag="pu")
                        nc.tensor.matmul(pu[:], sel, u16[ti][:el, ss],
                                         start=True, stop=True)
                        nc.vector.tensor_tensor(d1[:, ss], pu[:], brep[:, ss],
                                                OP.mult)
                    h = volp.tile([128, T], BF16, name="h", tag="d1", bufs=2)
                    for hb in range(2):
                        hs = slice(hb * 1024, (hb + 1) * 1024)
                        nc.vector.tensor_tensor_scan(h[:, hs], dA[:, hs], d1[:, hs],
                                                     0.0, OP.mult, OP.add)
                    yt = volp.tile([128, T], BF16, name="yt", tag="dA", bufs=3)
                    nc.gpsimd.tensor_tensor(yt[:], h[:], crep[:], OP.mult)
                    yts[k] = yt

                def issue_gsum(k):
                    nonlocal ytpA, ytpB
                    gi = k % 4
                    if gi == 0:
                        ytpA = ps_y.tile([32, 1024], FP32, name="ytpA", tag="ytpA")
                        ytpB = ps_y.tile([32, 1024], FP32, name="ytpB", tag="ytpB")
                    yt = yts.pop(k)
                    for sb in range(4):
                        ps = slice(sb * 512, (sb + 1) * 512)
                        ytp = ytpA if sb < 2 else ytpB
                        pp = slice((sb % 2) * 512, (sb % 2) * 512 + 512)
                        nc.tensor.matmul(ytp[:, pp], gs32[:, gi * 32:(gi + 1) * 32],
                                         yt[:, ps], start=(gi == 0), stop=(gi == 3))
                    if gi == 3:
                        g = k // 4
                        if g < 4:
                            dst = y_fm[0][32 * g:32 * (g + 1), :]
                        else:
                            dst = y_fm[1][32 * (g - 4):32 * (g - 3), :]
                        nc.vector.tensor_copy(dst[:, 0:1024], ytpA[:])
                        nc.vector.tensor_copy(dst[:, 1024:2048], ytpB[:])

                for k in range(NTIL + 1):
                    if k < NTIL:
                        issue_tile(k)
                    if k >= 1:
                        issue_gsum(k - 1)

                # ---- D-term, z-gate ----
                for ti, (eo, el) in enumerate(_etiles()):
                    nc.vector.scalar_tensor_tensor(y_fm[ti][:el, :], xc[ti][:el, :],
                                                   w_D[0:el, ti:ti + 1], y_fm[ti][:el, :],
                                                   OP.mult, OP.add)
                    nc.scalar.activation(z_t[ti][:el, :], z_t[ti][:el, :], AF.Silu)
                    nc.vector.tensor_tensor(y_fm[ti][:el, :], y_fm[ti][:el, :],
                                            z_t[ti][:el, :], OP.mult)

                if li < NL - 1:
                    # ---- out_proj partial + AllReduce + residual update ----
                    op_in = dramp.tile([DM, T], BF16, name="op_in", tag="op_in")
                    op_out = dramp.tile([DM, T], BF16, name="op_out", tag="op_out")
                    for mt in range(DTILES):
                        for f in range(T // 512):
                            fs = slice(f * 512, (f + 1) * 512)
                            pt = pmm([128, 512])
                            for ti, (eo, el) in enumerate(_etiles()):
                                nc.tensor.matmul(
                                    pt[:], w_out[0:el, ti * DM + mt * 128:ti * DM + (mt + 1) * 128],
                                    y_fm[ti][:el, fs], start=(ti == 0), stop=(ti == 1))
                            opf = st2.tile([128, 512], BF16, name="opf", tag="opf")
                            nc.scalar.copy(opf[:], pt[:])
                            nc.sync.dma_start(op_in[mt * 128:(mt + 1) * 128, fs], opf[:])
                    nc.gpsimd.collective_compute("AllReduce", OP.add,
                                                 replica_groups=[list(range(NC))],
                                                 ins=[op_in[:]], outs=[op_out[:]])
                    for j in range(DTILES):
                        for f in range(T // 512):
                            fs = slice(f * 512, (f + 1) * 512)
                            hs_f = st2.tile([128, 512], BF16, name="hs_f", tag="hs_f")
                            nc.sync.dma_start(hs_f[:], op_out[j * 128:(j + 1) * 128, fs])
                            nc.vector.tensor_tensor(residual[j][:, fs], residual[j][:, fs],
                                                    hs_f[:], OP.add)
                else:
                    # ---- last layer: token-major out_proj partial + residual/8,
                    #      ReduceScatter over tokens ----
                    rs_in = dramp.tile([T, DM], BF16, name="rs_in", tag="op_in")
                    rs_out = dramp.tile([TLOC, DM], BF16, name="rs_out", tag="rs_out")
                    for tb in range(T // 128):
                        ts = slice(tb * 128, (tb + 1) * 128)
                        pta = pmm([128, 512])
                        ptb = pmm([128, 256])
                        for pt, do, dl in [(pta, 0, 512), (ptb, 512, 256)]:
                            for ti, (eo, el) in enumerate(_etiles()):
                                nc.tensor.matmul(
                                    pt[:, 0:dl],
                                    y_fm[ti][:el, ts],
                                    w_out[0:el, ti * DM + do:ti * DM + do + dl],
                                    start=(ti == 0), stop=(ti == 1))
                        for j in range(DTILES):
                            pt = pta if j < 4 else ptb
                            co = j * 128 if j < 4 else (j - 4) * 128
                            nc.tensor.matmul(pt[:, co:co + 128],
                                             residual[j][:, ts], idt8[:],
                                             start=False, stop=True,
                                             skip_group_check=True)
                        rf = st2.tile([128, DM], BF16, name="rf", tag="rf")
                        nc.scalar.copy(rf[:, 0:512], pta[:])
                        nc.scalar.copy(rf[:, 512:DM], ptb[:])
                        nc.sync.dma_start(rs_in[ts, :], rf[:])
                    nc.gpsimd.collective_compute("ReduceScatter", OP.add,
                                                 replica_groups=[list(range(NC))],
                                                 ins=[rs_in[:]], outs=[rs_out[:]])

            # ================= final stage (this core's 256 tokens) =================
            nfw = nreps[:, 0:DM]
            nfb = nreps[:, DM:2 * DM]
            olw = nreps[:, 2 * DM:3 * DM]
            olb = nreps[:, 3 * DM:4 * DM]

            def ln_tm(src_ap, wrep, brep_, dst_ap):
                st = st2.tile([128, 1], FP32, name="lnt_st", tag="lnt_st", bufs=1)
                nc.vector.tensor_reduce(st[:], src_ap, axis=AX.X, op=OP.add)
                nc.scalar.activation(st[:], st[:], AF.Copy, scale=1.0 / DM)
                ot = st2.tile([128, DM], FP32, name="lnt_ot", tag="lnt_ot", bufs=1)
                nc.vector.tensor_scalar(ot[:], src_ap, st[:, 0:1], None, OP.subtract)
                sq = st2.tile([128, DM], FP32, name="lnt_sq", tag="lnt_sq", bufs=1)
                nc.vector.tensor_tensor(sq[:], ot[:], ot[:], OP.mult)
                v2 = st2.tile([128, 1], FP32, name="lnt_v2", tag="lnt_v2", bufs=1)
                nc.vector.tensor_reduce(v2[:], sq[:], axis=AX.X, op=OP.add)
                nc.scalar.activation(v2[:], v2[:], AF.Ln, bias=eps_ap[:], scale=1.0 / DM)
                nc.scalar.activation(v2[:], v2[:], AF.Exp, scale=-0.5)
                nc.vector.tensor_scalar(ot[:], ot[:], v2[:, 0:1], None, OP.mult)
                nc.vector.tensor_tensor(ot[:], ot[:], wrep, OP.mult)
                nc.vector.tensor_tensor(dst_ap, ot[:], brep_, OP.add)

            for i in range(2):
                mx = st2.tile([128, DM], BF16, name="mx", tag="opf")
                nc.sync.dma_start(mx[:], rs_out[i * 128:(i + 1) * 128, :])
                xst = st2.tile([128, DM], FP32, name="xst", tag="lnt_xs", bufs=1)
                nc.sync.dma_start(xst[:], xs_d[i * 128:(i + 1) * 128, :])
                gtt = st2.tile([128, DM], FP32, name="gtt", tag="lnt_gt", bufs=1)
                nc.sync.dma_start(gtt[:], gate_d[i * 128:(i + 1) * 128, :])
                mixed = st2.tile([128, DM], FP32, name="mixed", tag="lnt_mx", bufs=1)
                ln_tm(mx[:], nfw, nfb, mixed[:])
                ot2 = st2.tile([128, DM], FP32, name="ot2", tag="lnt_ot2", bufs=1)
                nc.vector.tensor_tensor(ot2[:], mixed[:], xst[:], OP.subtract)
                nc.vector.tensor_tensor(ot2[:], ot2[:], gtt[:], OP.mult)
                nc.vector.tensor_tensor(ot2[:], ot2[:], xst[:], OP.add)
                fin = st2.tile([128, DM], FP32, name="fin", tag="rf", bufs=1)
                ln_tm(ot2[:], olw, olb, fin[:])
                nc.sync.dma_start(out_d[i * 128:(i + 1) * 128, :], fin[:])

    nc.compile()
    return nc


def _pack_fm(arr, pad_to=128):
    arr = np.asarray(arr)
    if arr.ndim == 1:
        arr = arr[:, None]
    F, W = arr.shape
    nblk = (F + pad_to - 1) // pad_to
    outp = np.zeros((pad_to, nblk * W), dtype=arr.dtype)
    for b_ in range(nblk):
        blk = arr[b_ * pad_to:(b_ + 1) * pad_to]
        outp[:blk.shape[0], b_ * W:(b_ + 1) * W] = blk
    return outp


def _prep_inputs(inputs):
    f32 = np.float32
    x = np.ascontiguousarray(np.asarray(inputs["x"], f32).reshape(T, DM))
    bprob = np.ascontiguousarray(np.asarray(inputs["boundary_prob"], f32).reshape(1, T))
    A = -np.exp(np.asarray(inputs["A_log"], f32))
    a_scales = A[0, 0, :]
    # selk[r, j*128+m] = 1 iff r == 8j + m//16  (pick e-row band j, replicate x16)
    selk = np.zeros((128, 16 * 128), f32)
    for j in range(16):
        m = np.arange(128)
        selk[8 * j + m // 16, j * 128 + m] = 1.0
    # gs32[p, i*32+m] = 1 iff m == 8i + p//16  (sum over n into e-row band i)
    gs32 = np.zeros((128, 4 * 32), f32)
    for i in range(4):
        p = np.arange(128)
        gs32[p, i * 32 + 8 * i + p // 16] = 1.0
    avec = a_scales[np.arange(128) % 16][:, None].astype(f32)
    cw1 = np.asarray(inputs["ctrl_w1"], f32)
    w1x = _pack_fm(cw1[:, :DM].T)                       # [128, 6*DM]
    w1b = cw1[:, DM][None, :]                           # [1, DM]
    b1c = _pack_fm(np.asarray(inputs["ctrl_b1"], f32))  # [128, 6]
    w2t = _pack_fm(np.asarray(inputs["ctrl_w2"], f32).T)
    nreps = np.concatenate([
        np.broadcast_to(np.asarray(inputs["normf_w"], f32), (128, DM)),
        np.broadcast_to(np.asarray(inputs["normf_b"], f32), (128, DM)),
        np.broadcast_to(np.asarray(inputs["out_ln_w"], f32), (128, DM)),
        np.broadcast_to(np.asarray(inputs["out_ln_b"], f32), (128, DM)),
    ], axis=1)
    maps = []
    for c in range(NC):
        sl = slice(c * ELOC, (c + 1) * ELOC)
        w_in = np.stack([_pack_fm(
            np.concatenate([np.asarray(inputs["in_proj_w"][i])[sl],
                            np.asarray(inputs["in_proj_w"][i])[E + c * ELOC:E + (c + 1) * ELOC]],
                           axis=0).T.astype(f32))
            for i in range(NL)])
        w_xp = np.stack([_pack_fm(np.asarray(inputs["x_proj_w"][i], f32)[:, sl].T)
                         for i in range(NL)])
        w_dt = np.stack([
            np.concatenate([np.asarray(inputs["dt_proj_w"][i], f32)[sl].T,
                            np.zeros((16, ELOC), f32),
                            np.asarray(inputs["dt_proj_b"][i], f32)[None, sl]], axis=0)
            for i in range(NL)])
        w_out = np.stack([_pack_fm(np.asarray(inputs["out_proj_w"][i], f32)[:, sl].T)
                          for i in range(NL)])
        lnp = np.stack([_pack_fm(np.stack([np.asarray(inputs["ln_w"][i], f32),
                                           np.asarray(inputs["ln_b"][i], f32)], axis=1))
                        for i in range(NL)])
        tsl = slice(c * TLOC, (c + 1) * TLOC)
        maps.append({
            "x": x, "xs": np.ascontiguousarray(x[tsl]),
            "bps": np.ascontiguousarray(bprob[:, tsl]),
            "w_in": w_in,
            "conv_w": np.stack([_pack_fm(np.asarray(inputs["conv_w"][i], f32)[sl])
                                for i in range(NL)]),
            "conv_b": np.stack([_pack_fm(np.asarray(inputs["conv_b"][i], f32)[sl])
                                for i in range(NL)]),
            "w_xp": w_xp, "w_dt": w_dt, "w_out": w_out, "lnp": lnp,
            "ssmd": np.stack([_pack_fm(np.asarray(inputs["ssm_D"][i], f32)[sl])
                              for i in range(NL)]),
            "selk": selk, "gs32": gs32, "avec": avec,
            "w1x": w1x, "w1b": w1b, "b1c": b1c, "w2t": w2t,
            "nreps": nreps,
        })
    return maps


BF16_KEYS = ("w_in", "w_xp", "w_out", "w_dt", "selk", "gs32", "w1x", "w1b",
             "w2t", "bps", "nreps")


def kernel(**inputs):
    import ml_dtypes
    maps = _prep_inputs(inputs)
    A = -np.exp(np.asarray(inputs["A_log"], np.float32))
    a_scales = A[0, 0, :]
    for i in range(NL):
        assert np.allclose(A[i], np.broadcast_to(a_scales, (E, N)), rtol=1e-5, atol=1e-6), \
            "kernel assumes channel-independent A"
    if "nc" not in _CACHE:
        _CACHE["nc"] = _build()
    nc = _CACHE["nc"]
    for m in maps:
        for k in BF16_KEYS:
            m[k] = np.asarray(m[k], dtype=ml_dtypes.bfloat16)
    res = run_bass_kernel_spmd(nc, maps, list(range(NC)))
    kernel._res = res
    out = np.concatenate([np.asarray(res.results[c]["out"], np.float32)
                          for c in range(NC)], axis=0).reshape(B, L, DM)
    gate = np.concatenate([np.asarray(res.results[c]["gate"], np.float32)
                           for c in range(NC)], axis=0).reshape(B, L, DM)
    return out, gate
